# revision 10
# baseline (speedup 1.0000x reference)
"""Distributed Trainium2 Bass kernel for nn_CrossAttention.

Reference computation (per batch b):
    q = x @ Wq.T + bq          (N, C)       C = 1024, H = 16 heads, D = 64
    k = enc @ Wk.T + bk        (T, C)
    v = enc @ Wv.T + bv        (T, C)
    att = softmax(q.k / sqrt(D))   per head
    y = (att @ v) @ Wp.T + bp  (N, C)

Sharding (8 cores): core c = (batch b = c//2, head-group g = c%2).
Each core owns 8 heads (512 channels) of Q/K/V for one batch, computes
attention for those heads, and a *partial* output projection using the
512 matching columns of Wp.  Host sums the two partials per batch and
adds bp.  No inter-core communication.

Device-side layout is "feature on partitions" throughout:
    Q^T, K^T : (512, N)  channel-major, bf16.
    V        : (T, 512)  token-major bf16, with a ones column per head so
               attn@V also emits the softmax denominator.
    S^T = K Q^T : (128, n) fp32 in PSUM, exp(scale*s) on ScalarE -> P^T bf16.
    attn@V   : out = V'.T @ P^T -> (65, n) fp32 = [y^T ; denom].

Perf structure (v2):
  * All matmuls bf16 (FWL weight loads; f32r LDWEIGHTS was 2-3x slower).
  * Attention inner loop is ScalarE(exp)-bound at ~1.15us/t-block; PE has
    slack, so the normalize / out-proj chain is software-pipelined into
    the middle of LATER attention units so no engine ever stalls on it:
      - unit (nq,p) accumulates attn@V for all 16 t-blocks in one PSUM
        pair, then copies to SBUF accumulators av_sb[p] (2 DVE copies).
      - reciprocal of the denominators (DVE, 3.3us each) is emitted one
        unit later ("R stage"); y^T = av * (1/denom) plus the out-proj
        happens ~4 units later mid-unit ("F stage"), when everything is
        long ready.  This removes the ~5.6us/unit PE+ACT boundary stalls
        of v1 (which also HAM-rethrottled the PE clock every unit).
  * Rp (the 1/denom PE broadcast) uses the "pa" PSUM pool, not the score
    pool, so next unit's score matmuls never wait on the normalize chain.
"""

import numpy as np
from collections import deque
from contextlib import ExitStack

# ---------------------------------------------------------------- constants
B, N, T, C, H = 4, 2048, 2048, 1024, 16
G = 2                      # head groups (cores per batch)
N_CORES = 8
D = C // H                 # 64 head dim
HL = H // G                # 8 heads per core
CL = HL * D                # 512 local channels per core

_COMPILED = {}             # (aug_x, aug_e) -> compiled Bacc


def build(aug_x: bool, aug_e: bool, num_devices: int = N_CORES,
          n=N, t=T, c=C, hl=HL, d=D, repeat=1, proj_dtype="bf16"):
    """Build + compile the per-core SPMD program.  Parameterized so tests
    can build small versions for CoreSim (requires t == n)."""
    import concourse.mybir as mybir
    import concourse.tile as tile
    from concourse import bacc

    f32 = mybir.dt.float32
    bf16 = mybir.dt.bfloat16
    f32r = mybir.dt.float32r
    pdt = {"f32r": f32r, "bf16": bf16}[proj_dtype]
    EXP = mybir.ActivationFunctionType.Exp

    cl = hl * d
    dp1 = d + 1
    KC = c // 128                      # contraction chunks (proj)
    NCH = n // 512                     # n chunks of 512
    TB = t // 128                      # t blocks of 128
    MQ = cl // 128                     # q/k channel blocks (== head pairs)
    MO = c // 128                      # output channel blocks
    PAIRS = hl // 2
    NSLAB = TB // 4
    assert TB == NCH * 4 and MQ == PAIRS
    scale = 1.0 / float(np.sqrt(d))

    xrows = c + (1 if aug_x else 0)
    erows = c + (1 if aug_e else 0)
    kq_chunks = [(i * 128, 128) for i in range(KC)] + ([(c, 1)] if aug_x else [])
    ke_chunks = [(i * 128, 128) for i in range(KC)] + ([(c, 1)] if aug_e else [])

    nc = bacc.Bacc("TRN2", target_bir_lowering=False, debug=False,
                   enable_asserts=False, num_devices=num_devices)

    xt = nc.dram_tensor("xt", (xrows, n), pdt, kind="ExternalInput").ap()
    et = nc.dram_tensor("et", (erows, t), pdt, kind="ExternalInput").ap()
    wqt = nc.dram_tensor("wqt", (xrows, cl), pdt, kind="ExternalInput").ap()
    wkt = nc.dram_tensor("wkt", (erows, cl), pdt, kind="ExternalInput").ap()
    wvt = nc.dram_tensor("wvt", (erows, cl), pdt, kind="ExternalInput").ap()
    wpt = nc.dram_tensor("wpt", (cl, c), pdt, kind="ExternalInput").ap()
    ot = nc.dram_tensor("ot", (c, n), f32, kind="ExternalOutput").ap()

    def emit_body(tc):
        with ExitStack() as ctx:
            persist = ctx.enter_context(tc.tile_pool(name="persist", bufs=1))
            psum = ctx.enter_context(tc.tile_pool(name="psum", bufs=2,
                                                  space="PSUM"))
            spool = ctx.enter_context(tc.tile_pool(name="satt", bufs=2))

            qt = [persist.tile([128, n], bf16, name=f"qt{m}", tag=f"qt{m}")
                  for m in range(MQ)]
            kt = [persist.tile([128, t], bf16, name=f"kt{m}", tag=f"kt{m}")
                  for m in range(MQ)]
            vv = [persist.tile([128, hl * dp1], bf16, name=f"vv{i}",
                               tag=f"vv{i}") for i in range(TB)]
            wpt_sb = [persist.tile([128, c], pdt, name=f"wp{p}",
                                   tag=f"wp{p}") for p in range(PAIRS)]
            ones1 = persist.tile([dp1, d], pdt, name="ones1", tag="ones1")
            ones1f = persist.tile([dp1, d], f32, name="ones1f", tag="ones1f")
            nc.vector.memset(ones1f[d:dp1, :], 1.0)
            nc.vector.tensor_copy(ones1[d:dp1, :], ones1f[d:dp1, :])

            # --------------------------------------- phase KV (+ weights)
            kv_ctx = ExitStack()
            wk_pool = kv_ctx.enter_context(tc.tile_pool(name="wkp", bufs=1))
            es_pool = kv_ctx.enter_context(
                tc.tile_pool(name="esl", bufs=len(ke_chunks)))
            wk_sb, wv_sb = [], []

            def kv_iter(nt):
                """K^T and V for t-blocks 4nt..4nt+3."""
                es = []
                for ki, (off, sz) in enumerate(ke_chunks):
                    if nt == 0:
                        wkc = wk_pool.tile([sz, cl], pdt, name=f"wkc{ki}",
                                           tag=f"wkc{ki}")
                        nc.sync.dma_start(wkc, wkt[off:off + sz, :])
                        wk_sb.append(wkc)
                        wvc = wk_pool.tile([sz, cl], pdt, name=f"wvc{ki}",
                                           tag=f"wvc{ki}")
                        nc.sync.dma_start(wvc, wvt[off:off + sz, :])
                        wv_sb.append(wvc)
                    e = es_pool.tile([128, 512], pdt, name="es", tag="es")
                    nc.sync.dma_start(
                        e[:sz, :], et[off:off + sz, nt * 512:(nt + 1) * 512])
                    es.append(e)
                nk = len(ke_chunks)
                for mh in (range(0, MQ, 2) if MQ > 1 else [0]):
                    ms = [m for m in (mh, mh + 1) if m < MQ]
                    ps = [psum.tile([128, 512], f32, name=f"pk{m}", tag="pa")
                          for m in ms]
                    for ki, (off, sz) in enumerate(ke_chunks):
                        for j, m in enumerate(ms):
                            nc.tensor.matmul(
                                ps[j], wk_sb[ki][:, m * 128:(m + 1) * 128],
                                es[ki][:sz, :],
                                start=(ki == 0), stop=(ki == nk - 1))
                    for j, m in enumerate(ms):
                        nc.vector.tensor_copy(
                            kt[m][:, nt * 512:(nt + 1) * 512], ps[j])
                for th in (0, 2):
                    ps = [psum.tile([128, cl], f32, name=f"pv{tb}", tag="pa")
                          for tb in (th, th + 1)]
                    for ki, (off, sz) in enumerate(ke_chunks):
                        for j, tb in enumerate((th, th + 1)):
                            nc.tensor.matmul(
                                ps[j], es[ki][:sz, tb * 128:(tb + 1) * 128],
                                wv_sb[ki],
                                start=(ki == 0), stop=(ki == nk - 1))
                    for j, tb in enumerate((th, th + 1)):
                        ti = nt * 4 + tb
                        src = ps[j].rearrange("p (h e) -> p h e", h=hl)
                        dst = vv[ti].rearrange("p (h e) -> p h e", h=hl)
                        nc.vector.tensor_copy(dst[:, :, 0:d], src)
                        nc.vector.memset(dst[:, :, d:dp1], 1.0)

            # --------------------------------------- phase Q (+ weights)
            q_ctx = ExitStack()
            wq_pool = q_ctx.enter_context(tc.tile_pool(name="wqp", bufs=1))
            xs_pool = q_ctx.enter_context(
                tc.tile_pool(name="xsl", bufs=len(kq_chunks)))
            wq_sb = []

            def q_iter(nq):
                xs = []
                for ki, (off, sz) in enumerate(kq_chunks):
                    if nq == 0:
                        wqc = wq_pool.tile([sz, cl], pdt, name=f"wqc{ki}",
                                           tag=f"wqc{ki}")
                        nc.sync.dma_start(wqc, wqt[off:off + sz, :])
                        wq_sb.append(wqc)
                    x = xs_pool.tile([128, 512], pdt, name="xs", tag="xs")
                    nc.sync.dma_start(
                        x[:sz, :], xt[off:off + sz, nq * 512:(nq + 1) * 512])
                    xs.append(x)
                nk = len(kq_chunks)
                for mh in (range(0, MQ, 2) if MQ > 1 else [0]):
                    ms = [m for m in (mh, mh + 1) if m < MQ]
                    ps = [psum.tile([128, 512], f32, name=f"pq{m}", tag="pa")
                          for m in ms]
                    for ki, (off, sz) in enumerate(kq_chunks):
                        for j, m in enumerate(ms):
                            nc.tensor.matmul(
                                ps[j], wq_sb[ki][:, m * 128:(m + 1) * 128],
                                xs[ki][:sz, :],
                                start=(ki == 0), stop=(ki == nk - 1))
                    for j, m in enumerate(ms):
                        nc.vector.tensor_copy(
                            qt[m][:, nq * 512:(nq + 1) * 512], ps[j])

            # --------------------------------------- attention pieces
            def att_tblock(nq, p, avp, ti, first, last):
                """Scores + exp + attn@V for one t-block of head pair p."""
                h0, h1 = 2 * p, 2 * p + 1
                sc = psum.tile([128, 1024], f32, name="sc", tag="sc2")
                nc.tensor.matmul(
                    sc[:, 0:512],
                    kt[p][0:64, ti * 128:(ti + 1) * 128],
                    qt[p][0:64, nq * 512:(nq + 1) * 512],
                    start=True, stop=True)
                nc.tensor.matmul(
                    sc[:, 512:1024],
                    kt[p][64:128, ti * 128:(ti + 1) * 128],
                    qt[p][64:128, nq * 512:(nq + 1) * 512],
                    start=True, stop=True)
                pt = spool.tile([128, 1024], bf16, name="pt", tag="pt",
                                bufs=4)
                nc.scalar.activation(pt, sc, EXP, scale=scale)
                nc.tensor.matmul(
                    avp[0], vv[ti][:, h0 * dp1:(h0 + 1) * dp1],
                    pt[:, 0:512], start=first, stop=last)
                nc.tensor.matmul(
                    avp[1], vv[ti][:, h1 * dp1:(h1 + 1) * dp1],
                    pt[:, 512:1024], start=first, stop=last)

            def att_pair(nq, p, av, trange, first_slab):
                """Slab-granularity attention (window 1: interleaved with
                the KV/Q projection iterations, folds into av via DVE)."""
                trange = list(trange)
                avp = (psum.tile([dp1, 512], f32, name="avp0", tag="av0",
                                 bufs=1),
                       psum.tile([dp1, 512], f32, name="avp1", tag="av1",
                                 bufs=1))
                for ti in trange:
                    att_tblock(nq, p, avp, ti,
                               first=(ti == trange[0]), last=(ti == trange[-1]))
                for j in range(2):
                    if first_slab:
                        nc.vector.tensor_copy(av[j], avp[j])
                    else:
                        nc.vector.tensor_add(av[j], av[j], avp[j])

            def att_unit(nq, p, av, mids):
                """Full attention unit (window 2): one PSUM accumulation
                over all TB t-blocks, then 2 DVE copies into av.  mids maps
                slab index -> callback emitted between slabs (pipelined
                normalize / out-proj service)."""
                avp = (psum.tile([dp1, 512], f32, name="avp0", tag="av0",
                                 bufs=1),
                       psum.tile([dp1, 512], f32, name="avp1", tag="av1",
                                 bufs=1))
                for si in range(NSLAB):
                    for k2 in range(4):
                        ti = si * 4 + k2
                        att_tblock(nq, p, avp, ti,
                                   first=(ti == 0), last=(ti == TB - 1))
                    cb = mids.get(si)
                    if cb:
                        cb()
                for j in range(2):
                    nc.vector.tensor_copy(av[j], avp[j])

            def att_recip(av):
                """R stage: DVE reciprocals of the softmax denominators."""
                rcs = []
                for j in range(2):
                    rc = spool.tile([dp1, 512], pdt, name="rc", tag="rc",
                                    bufs=10)
                    with nc.allow_low_precision(reason="1/denom in bf16"):
                        nc.vector.reciprocal(rc[d:dp1, :], av[j][d:dp1, :])
                    rcs.append(rc)
                return rcs

            def att_finish(rcs, av, ytp):
                """F stage: PE broadcast of 1/denom, then y^T = av * R."""
                for j in range(2):
                    Rp = psum.tile([64, 512], f32, name="Rp", tag="pa")
                    nc.tensor.matmul(Rp, ones1[d:dp1, :], rcs[j][d:dp1, :],
                                     start=True, stop=True)
                    if j == 0:
                        nc.vector.tensor_mul(ytp[0:64, :], av[j][0:d, :], Rp)
                    else:
                        ytm = spool.tile([64, 512], pdt, name="ytm",
                                         tag="ytm", bufs=4)
                        nc.vector.tensor_mul(ytm, av[j][0:d, :], Rp)
                        nc.sync.dma_start(ytp[64:128, :], ytm)

            def out_proj(nq, yts):
                for m in range(MO):
                    po = psum.tile([128, 512], f32, name="po", tag="pa")
                    for p in range(PAIRS):
                        nc.tensor.matmul(
                            po, wpt_sb[p][:, m * 128:(m + 1) * 128], yts[p],
                            start=(p == 0), stop=(p == PAIRS - 1))
                    ob = spool.tile([128, 512], f32, name="ob", tag="ob",
                                    bufs=2)
                    nc.vector.tensor_copy(ob, po)
                    nc.sync.dma_start(ot[m * 128:(m + 1) * 128,
                                         nq * 512:(nq + 1) * 512], ob)

            # SBUF attn@V accumulators: three bank sets indexed by nq mod
            # 3, so unit (nq, p) never overwrites an accumulator whose
            # normalize (serviced up to ~2 units later) is still pending:
            # the same bank is only reused 3 n-chunks (12 units) later.
            NB = min(3, NCH)
            av_banks = [
                [(persist.tile([dp1, 512], f32, name=f"av{b}_{p}0",
                               tag=f"av{b}_{p}0"),
                  persist.tile([dp1, 512], f32, name=f"av{b}_{p}1",
                               tag=f"av{b}_{p}1"))
                 for p in range(PAIRS)]
                for b in range(NB)]

            # ---- pipelined normalize / out-proj service ----------------
            # Entries flow Rq --service_R--> Fq --service_F--> out_proj.
            # ucount is the emitted-unit counter; F only pops entries
            # whose reciprocal was emitted in a strictly earlier unit so
            # the Rp broadcast never head-of-line-blocks the PE FIFO
            # waiting on a DVE reciprocal.
            Rq = deque()               # (av, nq, p): needs reciprocal
            Fq = deque()               # (rcs, av, nq, p, r_u): needs finish
            yts_by = {nqi: [] for nqi in range(NCH)}
            ucount = [0]

            def service_R():
                if Rq:
                    av, nq2, p2 = Rq.popleft()
                    Fq.append((att_recip(av), av, nq2, p2, ucount[0]))

            def service_F(force=False):
                if Fq and (force or Fq[0][4] < ucount[0]):
                    rcs, av, nq2, p2, _ru = Fq.popleft()
                    ytp = spool.tile([128, 512], pdt, name="ytp", tag="ytp",
                                     bufs=PAIRS + 4)
                    att_finish(rcs, av, ytp)
                    yts_by[nq2].append(ytp)
                    if len(yts_by[nq2]) == PAIRS:
                        out_proj(nq2, yts_by[nq2])

            # window-1 hosts attention for these n-chunks (nq0 from nt=0,
            # nq1 catching up from nt=1), filling ScalarE while the PE
            # runs the projections; the rest are ACT-bound window-2 units.
            w1_nqs = [0] if NCH <= 2 else [0, 1]

            def av_for(nqi):
                return av_banks[nqi % NB]

            # ---- window 1: projections + w1 n-chunk attention -----------
            for nt in range(NCH):
                kv_iter(nt)
                q_iter(nt)
                if nt == 0:
                    for p in range(PAIRS):
                        nc.sync.dma_start(wpt_sb[p],
                                          wpt[p * 128:(p + 1) * 128, :])
                for p in range(PAIRS):
                    att_pair(0, p, av_for(0)[p], range(nt * 4, nt * 4 + 4),
                             first_slab=(nt == 0))
                if 1 in w1_nqs and nt >= 1:
                    slabs = [0, 1] if nt == 1 else [nt]
                    for si in slabs:
                        for p in range(PAIRS):
                            att_pair(1, p, av_for(1)[p],
                                     range(si * 4, si * 4 + 4),
                                     first_slab=(si == 0))
            for nqw in w1_nqs:
                for p in range(PAIRS):
                    Rq.append((av_for(nqw)[p], nqw, p))
            q_ctx.close()
            kv_ctx.close()

            # ---- window 2: remaining n-chunks with mid-unit service ----
            mids = {si: (service_R if si % 2 == 0 else service_F)
                    for si in range(NSLAB)}
            if NSLAB < 2:
                def both():
                    service_R()
                    service_F()
                mids = {0: both}
            for nq in range(w1_nqs[-1] + 1, NCH):
                for p in range(PAIRS):
                    ucount[0] += 1
                    att_unit(nq, p, av_for(nq)[p], mids)
                    Rq.append((av_for(nq)[p], nq, p))
            ucount[0] += 1
            while Rq:
                service_R()
            while Fq:
                service_F(force=True)

    with tile.TileContext(nc) as tc:
        for _rep in range(repeat):
            emit_body(tc)

    nc.compile()
    return nc


def _get_compiled(aug_x: bool, aug_e: bool):
    key = (aug_x, aug_e)
    if key not in _COMPILED:
        _COMPILED[key] = build(aug_x, aug_e)
    return _COMPILED[key]


def shard_inputs(x, enc, Wq, bq, Wk, bk, Wv, bv, Wp, aug_x, aug_e,
                 g_groups=G, cl=CL, proj_dtype="bf16"):
    if proj_dtype == "bf16":
        import ml_dtypes
        npdt = ml_dtypes.bfloat16
    else:
        npdt = np.float32
    in_maps = []
    n_cores = x.shape[0] * g_groups
    onesN = np.ones((1, x.shape[1]), np.float32)
    onesT = np.ones((1, enc.shape[1]), np.float32)
    for core in range(n_cores):
        b, g = divmod(core, g_groups)
        sl = slice(g * cl, (g + 1) * cl)
        xtc = x[b].T
        etc = enc[b].T
        wqtc = Wq[sl, :].T
        wktc = Wk[sl, :].T
        wvtc = Wv[sl, :].T
        if aug_x:
            xtc = np.concatenate([xtc, onesN], axis=0)
            wqtc = np.concatenate([wqtc, bq[sl][None, :]], axis=0)
        if aug_e:
            etc = np.concatenate([etc, onesT], axis=0)
            wktc = np.concatenate([wktc, bk[sl][None, :]], axis=0)
            wvtc = np.concatenate([wvtc, bv[sl][None, :]], axis=0)
        in_maps.append({
            "xt": np.ascontiguousarray(xtc, npdt),
            "et": np.ascontiguousarray(etc, npdt),
            "wqt": np.ascontiguousarray(wqtc, npdt),
            "wkt": np.ascontiguousarray(wktc, npdt),
            "wvt": np.ascontiguousarray(wvtc, npdt),
            "wpt": np.ascontiguousarray(Wp[:, sl].T, npdt),
        })
    return in_maps


def run_spmd(in_maps, nc=None, aug_x=False, aug_e=False, **kw):
    from concourse import bass_utils
    if nc is None:
        nc = _get_compiled(aug_x, aug_e)
    return bass_utils.run_bass_kernel_spmd(
        nc, in_maps, core_ids=list(range(len(in_maps))), **kw)


def kernel(**inputs):
    x = np.asarray(inputs["x"], np.float32)
    enc = np.asarray(inputs["encoder_output"], np.float32)
    Wq = np.asarray(inputs["Wq"], np.float32)
    bq = np.asarray(inputs["bq"], np.float32)
    Wk = np.asarray(inputs["Wk"], np.float32)
    bk = np.asarray(inputs["bk"], np.float32)
    Wv = np.asarray(inputs["Wv"], np.float32)
    bv = np.asarray(inputs["bv"], np.float32)
    Wp = np.asarray(inputs["Wp"], np.float32)
    bp = np.asarray(inputs["bp"], np.float32)

    aug_x = bool(np.any(bq))
    aug_e = bool(np.any(bk)) or bool(np.any(bv))
    nc = _get_compiled(aug_x, aug_e)
    in_maps = shard_inputs(x, enc, Wq, bq, Wk, bk, Wv, bv, Wp, aug_x, aug_e)
    res = run_spmd(in_maps, nc=nc)
    y = np.empty((B, N, C), np.float32)
    for b in range(B):
        y[b] = (res.results[2 * b]["ot"] +
                res.results[2 * b + 1]["ot"]).T + bp[None, :]
    return y


# revision 12
# speedup vs baseline: 1.1319x; 1.1319x over previous
"""Distributed Trainium2 Bass kernel for nn_CrossAttention.

Reference computation (per batch b):
    q = x @ Wq.T + bq          (N, C)       C = 1024, H = 16 heads, D = 64
    k = enc @ Wk.T + bk        (T, C)
    v = enc @ Wv.T + bv        (T, C)
    att = softmax(q.k / sqrt(D))   per head
    y = (att @ v) @ Wp.T + bp  (N, C)

Sharding (8 cores): core c = (batch b = c//2, head-group g = c%2).
Each core owns 8 heads (512 channels) of Q/K/V for one batch, computes
attention for those heads, and a *partial* output projection using the
512 matching columns of Wp.  Host sums the two partials per batch and
adds bp.  No inter-core communication.

Device-side layout is "feature on partitions" throughout:
    Q^T, K^T : (512, N)  channel-major, bf16.
    V        : (T, 512)  token-major bf16, with a ones column per head so
               attn@V also emits the softmax denominator.
    S^T = K Q^T : (128, n) fp32 in PSUM, exp(scale*s) on ScalarE -> P^T bf16.
    attn@V   : out = V'.T @ P^T -> (65, n) fp32 = [y^T ; denom].

Perf structure (v2):
  * All matmuls bf16 (FWL weight loads; f32r LDWEIGHTS was 2-3x slower).
  * Attention inner loop is ScalarE(exp)-bound at ~1.15us/t-block; PE has
    slack, so the normalize / out-proj chain is software-pipelined into
    the middle of LATER attention units so no engine ever stalls on it:
      - unit (nq,p) accumulates attn@V for all 16 t-blocks in one PSUM
        pair, then copies to SBUF accumulators av_sb[p] (2 DVE copies).
      - reciprocal of the denominators (DVE, 3.3us each) is emitted one
        unit later ("R stage"); y^T = av * (1/denom) plus the out-proj
        happens ~4 units later mid-unit ("F stage"), when everything is
        long ready.  This removes the ~5.6us/unit PE+ACT boundary stalls
        of v1 (which also HAM-rethrottled the PE clock every unit).
  * Rp (the 1/denom PE broadcast) uses the "pa" PSUM pool, not the score
    pool, so next unit's score matmuls never wait on the normalize chain.
"""

import numpy as np
from collections import deque
from contextlib import ExitStack

# ---------------------------------------------------------------- constants
B, N, T, C, H = 4, 2048, 2048, 1024, 16
G = 2                      # head groups (cores per batch)
N_CORES = 8
D = C // H                 # 64 head dim
HL = H // G                # 8 heads per core
CL = HL * D                # 512 local channels per core

_COMPILED = {}             # (aug_x, aug_e) -> compiled Bacc


def build(aug_x: bool, aug_e: bool, num_devices: int = N_CORES,
          n=N, t=T, c=C, hl=HL, d=D, repeat=1, proj_dtype="bf16"):
    """Build + compile the per-core SPMD program.  Parameterized so tests
    can build small versions for CoreSim (requires t == n)."""
    import concourse.mybir as mybir
    import concourse.tile as tile
    from concourse import bacc

    f32 = mybir.dt.float32
    bf16 = mybir.dt.bfloat16
    f32r = mybir.dt.float32r
    pdt = {"f32r": f32r, "bf16": bf16}[proj_dtype]
    EXP = mybir.ActivationFunctionType.Exp

    cl = hl * d
    dp1 = d + 1
    KC = c // 128                      # contraction chunks (proj)
    NCH = n // 512                     # n chunks of 512
    TB = t // 128                      # t blocks of 128
    MQ = cl // 128                     # q/k channel blocks (== head pairs)
    MO = c // 128                      # output channel blocks
    PAIRS = hl // 2
    NSLAB = TB // 4
    assert TB == NCH * 4 and MQ == PAIRS
    scale = 1.0 / float(np.sqrt(d))

    xrows = c + (1 if aug_x else 0)
    erows = c + (1 if aug_e else 0)
    kq_chunks = [(i * 128, 128) for i in range(KC)] + ([(c, 1)] if aug_x else [])
    ke_chunks = [(i * 128, 128) for i in range(KC)] + ([(c, 1)] if aug_e else [])

    nc = bacc.Bacc("TRN2", target_bir_lowering=False, debug=False,
                   enable_asserts=False, num_devices=num_devices)

    xt = nc.dram_tensor("xt", (xrows, n), pdt, kind="ExternalInput").ap()
    et = nc.dram_tensor("et", (erows, t), pdt, kind="ExternalInput").ap()
    wqt = nc.dram_tensor("wqt", (xrows, cl), pdt, kind="ExternalInput").ap()
    wkt = nc.dram_tensor("wkt", (erows, cl), pdt, kind="ExternalInput").ap()
    wvt = nc.dram_tensor("wvt", (erows, cl), pdt, kind="ExternalInput").ap()
    wpt = nc.dram_tensor("wpt", (cl, c), pdt, kind="ExternalInput").ap()
    ot = nc.dram_tensor("ot", (c, n), f32, kind="ExternalOutput").ap()

    def emit_body(tc):
        with ExitStack() as ctx:
            persist = ctx.enter_context(tc.tile_pool(name="persist", bufs=1))
            psum = ctx.enter_context(tc.tile_pool(name="psum", bufs=2,
                                                  space="PSUM"))
            spool = ctx.enter_context(tc.tile_pool(name="satt", bufs=2))

            qt = [persist.tile([128, n], bf16, name=f"qt{m}", tag=f"qt{m}")
                  for m in range(MQ)]
            kt = [persist.tile([128, t], bf16, name=f"kt{m}", tag=f"kt{m}")
                  for m in range(MQ)]
            vv = [persist.tile([128, hl * dp1], bf16, name=f"vv{i}",
                               tag=f"vv{i}") for i in range(TB)]
            wpt_sb = [persist.tile([128, c], pdt, name=f"wp{p}",
                                   tag=f"wp{p}") for p in range(PAIRS)]
            ones1 = persist.tile([dp1, d], pdt, name="ones1", tag="ones1")
            ones1f = persist.tile([dp1, d], f32, name="ones1f", tag="ones1f")
            nc.vector.memset(ones1f[d:dp1, :], 1.0)
            nc.vector.tensor_copy(ones1[d:dp1, :], ones1f[d:dp1, :])
            # warm the ACT exp table set during the DMA ramp so the first
            # real exp doesn't pay the ~2.7us ACT_TABLE_LOAD
            actw = persist.tile([1, 16], bf16, name="actw", tag="actw")
            nc.scalar.activation(actw, ones1f[d:d + 1, 0:16], EXP)

            # --------------------------------------- phase KV (+ weights)
            kv_ctx = ExitStack()
            wk_pool = kv_ctx.enter_context(tc.tile_pool(name="wkp", bufs=1))
            es_pool = kv_ctx.enter_context(
                tc.tile_pool(name="esl", bufs=len(ke_chunks)))
            wk_sb, wv_sb = [], []

            def kv_iter(nt):
                """K^T and V for t-blocks 4nt..4nt+3."""
                es = []
                for ki, (off, sz) in enumerate(ke_chunks):
                    if nt == 0:
                        wkc = wk_pool.tile([sz, cl], pdt, name=f"wkc{ki}",
                                           tag=f"wkc{ki}")
                        nc.sync.dma_start(wkc, wkt[off:off + sz, :])
                        wk_sb.append(wkc)
                        wvc = wk_pool.tile([sz, cl], pdt, name=f"wvc{ki}",
                                           tag=f"wvc{ki}")
                        nc.sync.dma_start(wvc, wvt[off:off + sz, :])
                        wv_sb.append(wvc)
                    e = es_pool.tile([128, 512], pdt, name="es", tag="es")
                    nc.sync.dma_start(
                        e[:sz, :], et[off:off + sz, nt * 512:(nt + 1) * 512])
                    es.append(e)
                nk = len(ke_chunks)
                for mh in (range(0, MQ, 2) if MQ > 1 else [0]):
                    ms = [m for m in (mh, mh + 1) if m < MQ]
                    ps = [psum.tile([128, 512], f32, name=f"pk{m}", tag="pa")
                          for m in ms]
                    for ki, (off, sz) in enumerate(ke_chunks):
                        for j, m in enumerate(ms):
                            nc.tensor.matmul(
                                ps[j], wk_sb[ki][:, m * 128:(m + 1) * 128],
                                es[ki][:sz, :],
                                start=(ki == 0), stop=(ki == nk - 1))
                    for j, m in enumerate(ms):
                        nc.vector.tensor_copy(
                            kt[m][:, nt * 512:(nt + 1) * 512], ps[j])
                for th in (0, 2):
                    ps = [psum.tile([128, cl], f32, name=f"pv{tb}", tag="pa")
                          for tb in (th, th + 1)]
                    for ki, (off, sz) in enumerate(ke_chunks):
                        for j, tb in enumerate((th, th + 1)):
                            nc.tensor.matmul(
                                ps[j], es[ki][:sz, tb * 128:(tb + 1) * 128],
                                wv_sb[ki],
                                start=(ki == 0), stop=(ki == nk - 1))
                    for j, tb in enumerate((th, th + 1)):
                        ti = nt * 4 + tb
                        src = ps[j].rearrange("p (h e) -> p h e", h=hl)
                        dst = vv[ti].rearrange("p (h e) -> p h e", h=hl)
                        nc.vector.tensor_copy(dst[:, :, 0:d], src)
                        nc.vector.memset(dst[:, :, d:dp1], 1.0)

            # --------------------------------------- phase Q (+ weights)
            q_ctx = ExitStack()
            wq_pool = q_ctx.enter_context(tc.tile_pool(name="wqp", bufs=1))
            xs_pool = q_ctx.enter_context(
                tc.tile_pool(name="xsl", bufs=len(kq_chunks)))
            wq_sb = []

            def q_iter(nq):
                xs = []
                for ki, (off, sz) in enumerate(kq_chunks):
                    if nq == 0:
                        wqc = wq_pool.tile([sz, cl], pdt, name=f"wqc{ki}",
                                           tag=f"wqc{ki}")
                        nc.sync.dma_start(wqc, wqt[off:off + sz, :])
                        wq_sb.append(wqc)
                    x = xs_pool.tile([128, 512], pdt, name="xs", tag="xs")
                    nc.sync.dma_start(
                        x[:sz, :], xt[off:off + sz, nq * 512:(nq + 1) * 512])
                    xs.append(x)
                nk = len(kq_chunks)
                for mh in (range(0, MQ, 2) if MQ > 1 else [0]):
                    ms = [m for m in (mh, mh + 1) if m < MQ]
                    ps = [psum.tile([128, 512], f32, name=f"pq{m}", tag="pa")
                          for m in ms]
                    for ki, (off, sz) in enumerate(kq_chunks):
                        for j, m in enumerate(ms):
                            nc.tensor.matmul(
                                ps[j], wq_sb[ki][:, m * 128:(m + 1) * 128],
                                xs[ki][:sz, :],
                                start=(ki == 0), stop=(ki == nk - 1))
                    for j, m in enumerate(ms):
                        nc.vector.tensor_copy(
                            qt[m][:, nq * 512:(nq + 1) * 512], ps[j])

            # --------------------------------------- attention pieces
            def att_tblock(nq, p, avp, ti, first, last):
                """Scores + exp + attn@V for one t-block of head pair p."""
                h0, h1 = 2 * p, 2 * p + 1
                sc = psum.tile([128, 1024], f32, name="sc", tag="sc2")
                nc.tensor.matmul(
                    sc[:, 0:512],
                    kt[p][0:64, ti * 128:(ti + 1) * 128],
                    qt[p][0:64, nq * 512:(nq + 1) * 512],
                    start=True, stop=True)
                nc.tensor.matmul(
                    sc[:, 512:1024],
                    kt[p][64:128, ti * 128:(ti + 1) * 128],
                    qt[p][64:128, nq * 512:(nq + 1) * 512],
                    start=True, stop=True)
                pt = spool.tile([128, 1024], bf16, name="pt", tag="pt",
                                bufs=4)
                nc.scalar.activation(pt, sc, EXP, scale=scale)
                nc.tensor.matmul(
                    avp[0], vv[ti][:, h0 * dp1:(h0 + 1) * dp1],
                    pt[:, 0:512], start=first, stop=last)
                nc.tensor.matmul(
                    avp[1], vv[ti][:, h1 * dp1:(h1 + 1) * dp1],
                    pt[:, 512:1024], start=first, stop=last)

            def att_pair(nq, p, av, trange, first_slab):
                """Slab-granularity attention (window 1: interleaved with
                the KV/Q projection iterations, folds into av via DVE)."""
                trange = list(trange)
                avp = (psum.tile([dp1, 512], f32, name="avp0", tag="av0",
                                 bufs=1),
                       psum.tile([dp1, 512], f32, name="avp1", tag="av1",
                                 bufs=1))
                for ti in trange:
                    att_tblock(nq, p, avp, ti,
                               first=(ti == trange[0]), last=(ti == trange[-1]))
                for j in range(2):
                    if first_slab:
                        nc.vector.tensor_copy(av[j], avp[j])
                    else:
                        nc.vector.tensor_add(av[j], av[j], avp[j])

            def att_unit(nq, p, av, mids):
                """Full attention unit (window 2): one PSUM accumulation
                over all TB t-blocks, then 2 DVE copies into av.  mids maps
                slab index -> callback emitted between slabs (pipelined
                normalize / out-proj service)."""
                avp = (psum.tile([dp1, 512], f32, name="avp0", tag="av0",
                                 bufs=1),
                       psum.tile([dp1, 512], f32, name="avp1", tag="av1",
                                 bufs=1))
                for si in range(NSLAB):
                    for k2 in range(4):
                        ti = si * 4 + k2
                        att_tblock(nq, p, avp, ti,
                                   first=(ti == 0), last=(ti == TB - 1))
                    cb = mids.get(si)
                    if cb:
                        cb()
                for j in range(2):
                    nc.vector.tensor_copy(av[j], avp[j])

            def att_recip(av):
                """R stage: DVE reciprocals of the softmax denominators."""
                rcs = []
                for j in range(2):
                    rc = spool.tile([dp1, 512], pdt, name="rc", tag="rc",
                                    bufs=10)
                    with nc.allow_low_precision(reason="1/denom in bf16"):
                        nc.vector.reciprocal(rc[d:dp1, :], av[j][d:dp1, :])
                    rcs.append(rc)
                return rcs

            def att_finish(rcs, av, ytp):
                """F stage: PE broadcast of 1/denom, then y^T = av * R."""
                for j in range(2):
                    Rp = psum.tile([64, 512], f32, name="Rp", tag="pa")
                    nc.tensor.matmul(Rp, ones1[d:dp1, :], rcs[j][d:dp1, :],
                                     start=True, stop=True)
                    if j == 0:
                        nc.vector.tensor_mul(ytp[0:64, :], av[j][0:d, :], Rp)
                    else:
                        ytm = spool.tile([64, 512], pdt, name="ytm",
                                         tag="ytm", bufs=4)
                        nc.vector.tensor_mul(ytm, av[j][0:d, :], Rp)
                        nc.sync.dma_start(ytp[64:128, :], ytm)

            def out_proj(nq, yts):
                for m in range(MO):
                    po = psum.tile([128, 512], f32, name="po", tag="pa")
                    for p in range(PAIRS):
                        nc.tensor.matmul(
                            po, wpt_sb[p][:, m * 128:(m + 1) * 128], yts[p],
                            start=(p == 0), stop=(p == PAIRS - 1))
                    ob = spool.tile([128, 512], f32, name="ob", tag="ob",
                                    bufs=2)
                    nc.vector.tensor_copy(ob, po)
                    nc.sync.dma_start(ot[m * 128:(m + 1) * 128,
                                         nq * 512:(nq + 1) * 512], ob)

            # SBUF attn@V accumulators: three bank sets indexed by nq mod
            # 3, so unit (nq, p) never overwrites an accumulator whose
            # normalize (serviced up to ~2 units later) is still pending:
            # the same bank is only reused 3 n-chunks (12 units) later.
            NB = min(3, NCH)
            av_banks = [
                [(persist.tile([dp1, 512], f32, name=f"av{b}_{p}0",
                               tag=f"av{b}_{p}0"),
                  persist.tile([dp1, 512], f32, name=f"av{b}_{p}1",
                               tag=f"av{b}_{p}1"))
                 for p in range(PAIRS)]
                for b in range(NB)]

            # ---- pipelined normalize / out-proj service ----------------
            # Entries flow Rq --service_R--> Fq --service_F--> out_proj.
            # ucount is the emitted-unit counter; F only pops entries
            # whose reciprocal was emitted in a strictly earlier unit so
            # the Rp broadcast never head-of-line-blocks the PE FIFO
            # waiting on a DVE reciprocal.
            Rq = deque()               # (av, nq, p): needs reciprocal
            Fq = deque()               # (rcs, av, nq, p, r_u): needs finish
            yts_by = {nqi: [] for nqi in range(NCH)}
            ucount = [0]

            def service_R():
                if Rq:
                    av, nq2, p2 = Rq.popleft()
                    Fq.append((att_recip(av), av, nq2, p2, ucount[0]))

            def service_F(force=False):
                if Fq and (force or Fq[0][4] < ucount[0]):
                    rcs, av, nq2, p2, _ru = Fq.popleft()
                    ytp = spool.tile([128, 512], pdt, name="ytp", tag="ytp",
                                     bufs=PAIRS + 4)
                    att_finish(rcs, av, ytp)
                    yts_by[nq2].append(ytp)
                    if len(yts_by[nq2]) == PAIRS:
                        out_proj(nq2, yts_by[nq2])

            # window-1 hosts attention for these n-chunks (nq0 from nt=0,
            # nq1 catching up from nt=1), filling ScalarE while the PE
            # runs the projections; the rest are ACT-bound window-2 units.
            # nq0 only: deeper w1 interleave loses to PE-FIFO head-of-line
            # blocking (avp matmuls gate later-emitted projection matmuls)
            w1_nqs = [0]

            def av_for(nqi):
                return av_banks[nqi % NB]

            # ---- window 1: projections + w1 n-chunk attention -----------
            for nt in range(NCH):
                kv_iter(nt)
                q_iter(nt)
                if nt == 0:
                    for p in range(PAIRS):
                        nc.sync.dma_start(wpt_sb[p],
                                          wpt[p * 128:(p + 1) * 128, :])
                for p in range(PAIRS):
                    att_pair(0, p, av_for(0)[p], range(nt * 4, nt * 4 + 4),
                             first_slab=(nt == 0))
                if 1 in w1_nqs and nt >= 1:
                    slabs = [0, 1] if nt == 1 else [nt]
                    for si in slabs:
                        for p in range(PAIRS):
                            att_pair(1, p, av_for(1)[p],
                                     range(si * 4, si * 4 + 4),
                                     first_slab=(si == 0))
            for nqw in w1_nqs:
                for p in range(PAIRS):
                    Rq.append((av_for(nqw)[p], nqw, p))
            q_ctx.close()
            kv_ctx.close()

            # ---- window 2: remaining n-chunks with mid-unit service ----
            mids = {si: (service_R if si % 2 == 0 else service_F)
                    for si in range(NSLAB)}
            if NSLAB < 2:
                def both():
                    service_R()
                    service_F()
                mids = {0: both}
            for nq in range(w1_nqs[-1] + 1, NCH):
                for p in range(PAIRS):
                    ucount[0] += 1
                    att_unit(nq, p, av_for(nq)[p], mids)
                    Rq.append((av_for(nq)[p], nq, p))
            ucount[0] += 1
            while Rq:
                service_R()
            while Fq:
                service_F(force=True)

    with tile.TileContext(nc) as tc:
        for _rep in range(repeat):
            emit_body(tc)

    nc.compile()
    return nc


def _get_compiled(aug_x: bool, aug_e: bool):
    key = (aug_x, aug_e)
    if key not in _COMPILED:
        _COMPILED[key] = build(aug_x, aug_e)
    return _COMPILED[key]


def shard_inputs(x, enc, Wq, bq, Wk, bk, Wv, bv, Wp, aug_x, aug_e,
                 g_groups=G, cl=CL, proj_dtype="bf16"):
    if proj_dtype == "bf16":
        import ml_dtypes
        npdt = ml_dtypes.bfloat16
    else:
        npdt = np.float32
    in_maps = []
    n_cores = x.shape[0] * g_groups
    onesN = np.ones((1, x.shape[1]), np.float32)
    onesT = np.ones((1, enc.shape[1]), np.float32)
    for core in range(n_cores):
        b, g = divmod(core, g_groups)
        sl = slice(g * cl, (g + 1) * cl)
        xtc = x[b].T
        etc = enc[b].T
        wqtc = Wq[sl, :].T
        wktc = Wk[sl, :].T
        wvtc = Wv[sl, :].T
        if aug_x:
            xtc = np.concatenate([xtc, onesN], axis=0)
            wqtc = np.concatenate([wqtc, bq[sl][None, :]], axis=0)
        if aug_e:
            etc = np.concatenate([etc, onesT], axis=0)
            wktc = np.concatenate([wktc, bk[sl][None, :]], axis=0)
            wvtc = np.concatenate([wvtc, bv[sl][None, :]], axis=0)
        in_maps.append({
            "xt": np.ascontiguousarray(xtc, npdt),
            "et": np.ascontiguousarray(etc, npdt),
            "wqt": np.ascontiguousarray(wqtc, npdt),
            "wkt": np.ascontiguousarray(wktc, npdt),
            "wvt": np.ascontiguousarray(wvtc, npdt),
            "wpt": np.ascontiguousarray(Wp[:, sl].T, npdt),
        })
    return in_maps


def run_spmd(in_maps, nc=None, aug_x=False, aug_e=False, **kw):
    from concourse import bass_utils
    if nc is None:
        nc = _get_compiled(aug_x, aug_e)
    return bass_utils.run_bass_kernel_spmd(
        nc, in_maps, core_ids=list(range(len(in_maps))), **kw)


def kernel(**inputs):
    x = np.asarray(inputs["x"], np.float32)
    enc = np.asarray(inputs["encoder_output"], np.float32)
    Wq = np.asarray(inputs["Wq"], np.float32)
    bq = np.asarray(inputs["bq"], np.float32)
    Wk = np.asarray(inputs["Wk"], np.float32)
    bk = np.asarray(inputs["bk"], np.float32)
    Wv = np.asarray(inputs["Wv"], np.float32)
    bv = np.asarray(inputs["bv"], np.float32)
    Wp = np.asarray(inputs["Wp"], np.float32)
    bp = np.asarray(inputs["bp"], np.float32)

    aug_x = bool(np.any(bq))
    aug_e = bool(np.any(bk)) or bool(np.any(bv))
    nc = _get_compiled(aug_x, aug_e)
    in_maps = shard_inputs(x, enc, Wq, bq, Wk, bk, Wv, bv, Wp, aug_x, aug_e)
    res = run_spmd(in_maps, nc=nc)
    y = np.empty((B, N, C), np.float32)
    for b in range(B):
        y[b] = (res.results[2 * b]["ot"] +
                res.results[2 * b + 1]["ot"]).T + bp[None, :]
    return y


# revision 15
# speedup vs baseline: 1.1534x; 1.0190x over previous
"""Distributed Trainium2 Bass kernel for nn_CrossAttention.

Reference computation (per batch b):
    q = x @ Wq.T + bq          (N, C)       C = 1024, H = 16 heads, D = 64
    k = enc @ Wk.T + bk        (T, C)
    v = enc @ Wv.T + bv        (T, C)
    att = softmax(q.k / sqrt(D))   per head
    y = (att @ v) @ Wp.T + bp  (N, C)

Sharding (8 cores): core c = (batch b = c//2, head-group g = c%2).
Each core owns 8 heads (512 channels) of Q/K/V for one batch, computes
attention for those heads, and a *partial* output projection using the
512 matching columns of Wp.  Host sums the two partials per batch and
adds bp.  No inter-core communication.

Device-side layout is "feature on partitions" throughout:
    Q^T, K^T : (512, N)  channel-major, bf16.
    V        : (T, 512)  token-major bf16, with a ones column per head so
               attn@V also emits the softmax denominator.
    S^T = K Q^T : (128, n) fp32 in PSUM, exp(scale*s) on ScalarE -> P^T bf16.
    attn@V   : out = V'.T @ P^T -> (65, n) fp32 = [y^T ; denom].

Perf structure (v2):
  * All matmuls bf16 (FWL weight loads; f32r LDWEIGHTS was 2-3x slower).
  * Attention inner loop is ScalarE(exp)-bound at ~1.15us/t-block; PE has
    slack, so the normalize / out-proj chain is software-pipelined into
    the middle of LATER attention units so no engine ever stalls on it:
      - unit (nq,p) accumulates attn@V for all 16 t-blocks in one PSUM
        pair, then copies to SBUF accumulators av_sb[p] (2 DVE copies).
      - reciprocal of the denominators (DVE, 3.3us each) is emitted one
        unit later ("R stage"); y^T = av * (1/denom) plus the out-proj
        happens ~4 units later mid-unit ("F stage"), when everything is
        long ready.  This removes the ~5.6us/unit PE+ACT boundary stalls
        of v1 (which also HAM-rethrottled the PE clock every unit).
  * Rp (the 1/denom PE broadcast) uses the "pa" PSUM pool, not the score
    pool, so next unit's score matmuls never wait on the normalize chain.
"""

import numpy as np
from collections import deque
from contextlib import ExitStack

# ---------------------------------------------------------------- constants
B, N, T, C, H = 4, 2048, 2048, 1024, 16
G = 2                      # head groups (cores per batch)
N_CORES = 8
D = C // H                 # 64 head dim
HL = H // G                # 8 heads per core
CL = HL * D                # 512 local channels per core

_COMPILED = {}             # (aug_x, aug_e) -> compiled Bacc


def build(aug_x: bool, aug_e: bool, num_devices: int = N_CORES,
          n=N, t=T, c=C, hl=HL, d=D, repeat=1, proj_dtype="bf16"):
    """Build + compile the per-core SPMD program.  Parameterized so tests
    can build small versions for CoreSim (requires t == n)."""
    import concourse.mybir as mybir
    import concourse.tile as tile
    from concourse import bacc

    f32 = mybir.dt.float32
    bf16 = mybir.dt.bfloat16
    f32r = mybir.dt.float32r
    pdt = {"f32r": f32r, "bf16": bf16}[proj_dtype]
    EXP = mybir.ActivationFunctionType.Exp

    cl = hl * d
    dp1 = d + 1
    KC = c // 128                      # contraction chunks (proj)
    NCH = n // 512                     # n chunks of 512
    TB = t // 128                      # t blocks of 128
    MQ = cl // 128                     # q/k channel blocks (== head pairs)
    MO = c // 128                      # output channel blocks
    PAIRS = hl // 2
    NSLAB = TB // 4
    assert TB == NCH * 4 and MQ == PAIRS
    scale = 1.0 / float(np.sqrt(d))

    xrows = c + (1 if aug_x else 0)
    erows = c + (1 if aug_e else 0)
    kq_chunks = [(i * 128, 128) for i in range(KC)] + ([(c, 1)] if aug_x else [])
    ke_chunks = [(i * 128, 128) for i in range(KC)] + ([(c, 1)] if aug_e else [])

    nc = bacc.Bacc("TRN2", target_bir_lowering=False, debug=False,
                   enable_asserts=False, num_devices=num_devices)

    xt = nc.dram_tensor("xt", (xrows, n), pdt, kind="ExternalInput").ap()
    et = nc.dram_tensor("et", (erows, t), pdt, kind="ExternalInput").ap()
    wqt = nc.dram_tensor("wqt", (xrows, cl), pdt, kind="ExternalInput").ap()
    wkt = nc.dram_tensor("wkt", (erows, cl), pdt, kind="ExternalInput").ap()
    wvt = nc.dram_tensor("wvt", (erows, cl), pdt, kind="ExternalInput").ap()
    wpt = nc.dram_tensor("wpt", (cl, c), pdt, kind="ExternalInput").ap()
    ot = nc.dram_tensor("ot", (c, n), f32, kind="ExternalOutput").ap()

    def emit_body(tc):
        with ExitStack() as ctx:
            persist = ctx.enter_context(tc.tile_pool(name="persist", bufs=1))
            psum = ctx.enter_context(tc.tile_pool(name="psum", bufs=2,
                                                  space="PSUM"))
            spool = ctx.enter_context(tc.tile_pool(name="satt", bufs=2))

            qt = [persist.tile([128, n], bf16, name=f"qt{m}", tag=f"qt{m}")
                  for m in range(MQ)]
            kt = [persist.tile([128, t], bf16, name=f"kt{m}", tag=f"kt{m}")
                  for m in range(MQ)]
            vv = [persist.tile([128, hl * dp1], bf16, name=f"vv{i}",
                               tag=f"vv{i}") for i in range(TB)]
            wpt_sb = [persist.tile([128, c], pdt, name=f"wp{p}",
                                   tag=f"wp{p}") for p in range(PAIRS)]
            ones1 = persist.tile([dp1, d], pdt, name="ones1", tag="ones1")
            ones1f = persist.tile([dp1, d], f32, name="ones1f", tag="ones1f")
            nc.vector.memset(ones1f[d:dp1, :], 1.0)
            nc.vector.tensor_copy(ones1[d:dp1, :], ones1f[d:dp1, :])
            # warm the ACT exp table set during the DMA ramp so the first
            # real exp doesn't pay the ~2.7us ACT_TABLE_LOAD
            actw = persist.tile([1, 16], bf16, name="actw", tag="actw")
            nc.scalar.activation(actw, ones1f[d:d + 1, 0:16], EXP)

            # --------------------------------------- phase KV (+ weights)
            kv_ctx = ExitStack()
            wk_pool = kv_ctx.enter_context(tc.tile_pool(name="wkp", bufs=1))
            es_pool = kv_ctx.enter_context(
                tc.tile_pool(name="esl", bufs=len(ke_chunks)))
            wk_sb, wv_sb = [], []

            def kv_iter(nt):
                """K^T and V for t-blocks 4nt..4nt+3."""
                es = []
                for ki, (off, sz) in enumerate(ke_chunks):
                    if nt == 0:
                        wkc = wk_pool.tile([sz, cl], pdt, name=f"wkc{ki}",
                                           tag=f"wkc{ki}")
                        nc.sync.dma_start(wkc, wkt[off:off + sz, :])
                        wk_sb.append(wkc)
                        wvc = wk_pool.tile([sz, cl], pdt, name=f"wvc{ki}",
                                           tag=f"wvc{ki}")
                        nc.sync.dma_start(wvc, wvt[off:off + sz, :])
                        wv_sb.append(wvc)
                    e = es_pool.tile([128, 512], pdt, name="es", tag="es")
                    nc.sync.dma_start(
                        e[:sz, :], et[off:off + sz, nt * 512:(nt + 1) * 512])
                    es.append(e)
                nk = len(ke_chunks)
                for mh in (range(0, MQ, 2) if MQ > 1 else [0]):
                    ms = [m for m in (mh, mh + 1) if m < MQ]
                    ps = [psum.tile([128, 512], f32, name=f"pk{m}", tag="pa")
                          for m in ms]
                    for ki, (off, sz) in enumerate(ke_chunks):
                        for j, m in enumerate(ms):
                            nc.tensor.matmul(
                                ps[j], wk_sb[ki][:, m * 128:(m + 1) * 128],
                                es[ki][:sz, :],
                                start=(ki == 0), stop=(ki == nk - 1))
                    for j, m in enumerate(ms):
                        nc.vector.tensor_copy(
                            kt[m][:, nt * 512:(nt + 1) * 512], ps[j])
                for th in (0, 2):
                    ps = [psum.tile([128, cl], f32, name=f"pv{tb}", tag="pa")
                          for tb in (th, th + 1)]
                    for ki, (off, sz) in enumerate(ke_chunks):
                        for j, tb in enumerate((th, th + 1)):
                            nc.tensor.matmul(
                                ps[j], es[ki][:sz, tb * 128:(tb + 1) * 128],
                                wv_sb[ki],
                                start=(ki == 0), stop=(ki == nk - 1))
                    for j, tb in enumerate((th, th + 1)):
                        ti = nt * 4 + tb
                        src = ps[j].rearrange("p (h e) -> p h e", h=hl)
                        dst = vv[ti].rearrange("p (h e) -> p h e", h=hl)
                        nc.vector.tensor_copy(dst[:, :, 0:d], src)
                        nc.vector.memset(dst[:, :, d:dp1], 1.0)

            # --------------------------------------- phase Q (+ weights)
            q_ctx = ExitStack()
            wq_pool = q_ctx.enter_context(tc.tile_pool(name="wqp", bufs=1))
            xs_pool = q_ctx.enter_context(
                tc.tile_pool(name="xsl", bufs=len(kq_chunks)))
            wq_sb = []

            def q_iter(nq):
                xs = []
                for ki, (off, sz) in enumerate(kq_chunks):
                    if nq == 0:
                        wqc = wq_pool.tile([sz, cl], pdt, name=f"wqc{ki}",
                                           tag=f"wqc{ki}")
                        nc.sync.dma_start(wqc, wqt[off:off + sz, :])
                        wq_sb.append(wqc)
                    x = xs_pool.tile([128, 512], pdt, name="xs", tag="xs")
                    nc.sync.dma_start(
                        x[:sz, :], xt[off:off + sz, nq * 512:(nq + 1) * 512])
                    xs.append(x)
                nk = len(kq_chunks)
                for mh in (range(0, MQ, 2) if MQ > 1 else [0]):
                    ms = [m for m in (mh, mh + 1) if m < MQ]
                    ps = [psum.tile([128, 512], f32, name=f"pq{m}", tag="pa")
                          for m in ms]
                    for ki, (off, sz) in enumerate(kq_chunks):
                        for j, m in enumerate(ms):
                            nc.tensor.matmul(
                                ps[j], wq_sb[ki][:, m * 128:(m + 1) * 128],
                                xs[ki][:sz, :],
                                start=(ki == 0), stop=(ki == nk - 1))
                    for j, m in enumerate(ms):
                        nc.vector.tensor_copy(
                            qt[m][:, nq * 512:(nq + 1) * 512], ps[j])

            # --------------------------------------- attention pieces
            def att_tblock(nq, p, avp, ti, first, last):
                """Scores + exp + attn@V for one t-block of head pair p."""
                h0, h1 = 2 * p, 2 * p + 1
                sc = psum.tile([128, 1024], f32, name="sc", tag="sc2")
                nc.tensor.matmul(
                    sc[:, 0:512],
                    kt[p][0:64, ti * 128:(ti + 1) * 128],
                    qt[p][0:64, nq * 512:(nq + 1) * 512],
                    start=True, stop=True)
                nc.tensor.matmul(
                    sc[:, 512:1024],
                    kt[p][64:128, ti * 128:(ti + 1) * 128],
                    qt[p][64:128, nq * 512:(nq + 1) * 512],
                    start=True, stop=True)
                pt = spool.tile([128, 1024], bf16, name="pt", tag="pt",
                                bufs=4)
                nc.scalar.activation(pt, sc, EXP, scale=scale)
                nc.tensor.matmul(
                    avp[0], vv[ti][:, h0 * dp1:(h0 + 1) * dp1],
                    pt[:, 0:512], start=first, stop=last)
                nc.tensor.matmul(
                    avp[1], vv[ti][:, h1 * dp1:(h1 + 1) * dp1],
                    pt[:, 512:1024], start=first, stop=last)

            def att_pair(nq, p, av, trange, first_slab):
                """Slab-granularity attention (window 1: interleaved with
                the KV/Q projection iterations, folds into av via DVE)."""
                trange = list(trange)
                avp = (psum.tile([dp1, 512], f32, name="avp0", tag="av0",
                                 bufs=1),
                       psum.tile([dp1, 512], f32, name="avp1", tag="av1",
                                 bufs=1))
                for ti in trange:
                    att_tblock(nq, p, avp, ti,
                               first=(ti == trange[0]), last=(ti == trange[-1]))
                for j in range(2):
                    if first_slab:
                        nc.vector.tensor_copy(av[j], avp[j])
                    else:
                        nc.vector.tensor_add(av[j], av[j], avp[j])

            def att_unit(nq, p, av, mids):
                """Full attention unit (window 2): one PSUM accumulation
                over all TB t-blocks, then 2 DVE copies into av.  mids maps
                slab index -> callback emitted between slabs (pipelined
                normalize / out-proj service)."""
                avp = (psum.tile([dp1, 512], f32, name="avp0", tag="av0",
                                 bufs=1),
                       psum.tile([dp1, 512], f32, name="avp1", tag="av1",
                                 bufs=1))
                for si in range(NSLAB):
                    for k2 in range(4):
                        ti = si * 4 + k2
                        att_tblock(nq, p, avp, ti,
                                   first=(ti == 0), last=(ti == TB - 1))
                    cb = mids.get(si)
                    if cb:
                        cb()
                for j in range(2):
                    nc.vector.tensor_copy(av[j], avp[j])

            def att_recip(av):
                """R stage: DVE reciprocals of the softmax denominators."""
                rcs = []
                for j in range(2):
                    rc = spool.tile([dp1, 512], pdt, name="rc", tag="rc",
                                    bufs=10)
                    with nc.allow_low_precision(reason="1/denom in bf16"):
                        nc.vector.reciprocal(rc[d:dp1, :], av[j][d:dp1, :])
                    rcs.append(rc)
                return rcs

            def att_finish(rcs, av, ytp):
                """F stage: PE broadcast of 1/denom, then y^T = av * R."""
                for j in range(2):
                    Rp = psum.tile([64, 512], f32, name="Rp", tag="pa")
                    nc.tensor.matmul(Rp, ones1[d:dp1, :], rcs[j][d:dp1, :],
                                     start=True, stop=True)
                    if j == 0:
                        nc.vector.tensor_mul(ytp[0:64, :], av[j][0:d, :], Rp)
                    else:
                        ytm = spool.tile([64, 512], pdt, name="ytm",
                                         tag="ytm", bufs=4)
                        nc.vector.tensor_mul(ytm, av[j][0:d, :], Rp)
                        nc.sync.dma_start(ytp[64:128, :], ytm)

            def out_proj(nq, yts):
                for m in range(MO):
                    po = psum.tile([128, 512], f32, name="po", tag="pa")
                    for p in range(PAIRS):
                        nc.tensor.matmul(
                            po, wpt_sb[p][:, m * 128:(m + 1) * 128], yts[p],
                            start=(p == 0), stop=(p == PAIRS - 1))
                    ob = spool.tile([128, 512], f32, name="ob", tag="ob",
                                    bufs=2)
                    nc.vector.tensor_copy(ob, po)
                    nc.sync.dma_start(ot[m * 128:(m + 1) * 128,
                                         nq * 512:(nq + 1) * 512], ob)

            # SBUF attn@V accumulators: three bank sets indexed by nq mod
            # 3, so unit (nq, p) never overwrites an accumulator whose
            # normalize (serviced up to ~2 units later) is still pending:
            # the same bank is only reused 3 n-chunks (12 units) later.
            NB = min(3, NCH)
            av_banks = [
                [(persist.tile([dp1, 512], f32, name=f"av{b}_{p}0",
                               tag=f"av{b}_{p}0"),
                  persist.tile([dp1, 512], f32, name=f"av{b}_{p}1",
                               tag=f"av{b}_{p}1"))
                 for p in range(PAIRS)]
                for b in range(NB)]

            # ---- pipelined normalize / out-proj service ----------------
            # Entries flow Rq --service_R--> Fq --service_F--> out_proj.
            # ucount is the emitted-unit counter; F only pops entries
            # whose reciprocal was emitted in a strictly earlier unit so
            # the Rp broadcast never head-of-line-blocks the PE FIFO
            # waiting on a DVE reciprocal.
            Rq = deque()               # (av, nq, p): needs reciprocal
            Fq = deque()               # (rcs, av, nq, p, r_u): needs finish
            yts_by = {nqi: [] for nqi in range(NCH)}
            ucount = [0]

            def service_R():
                if Rq:
                    av, nq2, p2 = Rq.popleft()
                    Fq.append((att_recip(av), av, nq2, p2, ucount[0]))

            def service_F(force=False):
                if Fq:
                    rcs, av, nq2, p2, _ru = Fq.popleft()
                    ytp = spool.tile([128, 512], pdt, name="ytp", tag="ytp",
                                     bufs=PAIRS + 4)
                    att_finish(rcs, av, ytp)
                    yts_by[nq2].append(ytp)
                    if len(yts_by[nq2]) == PAIRS:
                        out_proj(nq2, yts_by[nq2])

            # window-1 hosts attention for these n-chunks (nq0 from nt=0,
            # nq1 catching up from nt=1), filling ScalarE while the PE
            # runs the projections; the rest are ACT-bound window-2 units.
            # nq0 only: deeper w1 interleave loses to PE-FIFO head-of-line
            # blocking (avp matmuls gate later-emitted projection matmuls)
            w1_nqs = [0]

            def av_for(nqi):
                return av_banks[nqi % NB]

            # ---- window 1: projections + w1 n-chunk attention -----------
            for nt in range(NCH):
                kv_iter(nt)
                q_iter(nt)
                if nt == 0:
                    for p in range(PAIRS):
                        nc.sync.dma_start(wpt_sb[p],
                                          wpt[p * 128:(p + 1) * 128, :])
                for p in range(PAIRS):
                    att_pair(0, p, av_for(0)[p], range(nt * 4, nt * 4 + 4),
                             first_slab=(nt == 0))
                    if nt == NCH - 1:
                        # recip immediately (spread across the nt tail);
                        # consumed by F stages in early window-2 units
                        Fq.append((att_recip(av_for(0)[p]), av_for(0)[p],
                                   0, p, 0))
            q_ctx.close()
            kv_ctx.close()

            # ---- window 2: remaining n-chunks with mid-unit service ----
            f_slab = min(2, NSLAB - 1)
            mids = {0: service_R, f_slab: service_F}
            if NSLAB < 2:
                def both():
                    service_R()
                    service_F()
                mids = {0: both}
            for nq in range(w1_nqs[-1] + 1, NCH):
                for p in range(PAIRS):
                    ucount[0] += 1
                    att_unit(nq, p, av_for(nq)[p], mids)
                    Rq.append((av_for(nq)[p], nq, p))
            ucount[0] += 1
            while Rq:
                service_R()
            while Fq:
                service_F(force=True)

    with tile.TileContext(nc) as tc:
        for _rep in range(repeat):
            emit_body(tc)

    nc.compile()
    return nc


def _get_compiled(aug_x: bool, aug_e: bool):
    key = (aug_x, aug_e)
    if key not in _COMPILED:
        _COMPILED[key] = build(aug_x, aug_e)
    return _COMPILED[key]


def shard_inputs(x, enc, Wq, bq, Wk, bk, Wv, bv, Wp, aug_x, aug_e,
                 g_groups=G, cl=CL, proj_dtype="bf16"):
    if proj_dtype == "bf16":
        import ml_dtypes
        npdt = ml_dtypes.bfloat16
    else:
        npdt = np.float32
    in_maps = []
    n_cores = x.shape[0] * g_groups
    onesN = np.ones((1, x.shape[1]), np.float32)
    onesT = np.ones((1, enc.shape[1]), np.float32)
    for core in range(n_cores):
        b, g = divmod(core, g_groups)
        sl = slice(g * cl, (g + 1) * cl)
        xtc = x[b].T
        etc = enc[b].T
        wqtc = Wq[sl, :].T
        wktc = Wk[sl, :].T
        wvtc = Wv[sl, :].T
        if aug_x:
            xtc = np.concatenate([xtc, onesN], axis=0)
            wqtc = np.concatenate([wqtc, bq[sl][None, :]], axis=0)
        if aug_e:
            etc = np.concatenate([etc, onesT], axis=0)
            wktc = np.concatenate([wktc, bk[sl][None, :]], axis=0)
            wvtc = np.concatenate([wvtc, bv[sl][None, :]], axis=0)
        in_maps.append({
            "xt": np.ascontiguousarray(xtc, npdt),
            "et": np.ascontiguousarray(etc, npdt),
            "wqt": np.ascontiguousarray(wqtc, npdt),
            "wkt": np.ascontiguousarray(wktc, npdt),
            "wvt": np.ascontiguousarray(wvtc, npdt),
            "wpt": np.ascontiguousarray(Wp[:, sl].T, npdt),
        })
    return in_maps


def run_spmd(in_maps, nc=None, aug_x=False, aug_e=False, **kw):
    from concourse import bass_utils
    if nc is None:
        nc = _get_compiled(aug_x, aug_e)
    return bass_utils.run_bass_kernel_spmd(
        nc, in_maps, core_ids=list(range(len(in_maps))), **kw)


def kernel(**inputs):
    x = np.asarray(inputs["x"], np.float32)
    enc = np.asarray(inputs["encoder_output"], np.float32)
    Wq = np.asarray(inputs["Wq"], np.float32)
    bq = np.asarray(inputs["bq"], np.float32)
    Wk = np.asarray(inputs["Wk"], np.float32)
    bk = np.asarray(inputs["bk"], np.float32)
    Wv = np.asarray(inputs["Wv"], np.float32)
    bv = np.asarray(inputs["bv"], np.float32)
    Wp = np.asarray(inputs["Wp"], np.float32)
    bp = np.asarray(inputs["bp"], np.float32)

    aug_x = bool(np.any(bq))
    aug_e = bool(np.any(bk)) or bool(np.any(bv))
    nc = _get_compiled(aug_x, aug_e)
    in_maps = shard_inputs(x, enc, Wq, bq, Wk, bk, Wv, bv, Wp, aug_x, aug_e)
    res = run_spmd(in_maps, nc=nc)
    y = np.empty((B, N, C), np.float32)
    for b in range(B):
        y[b] = (res.results[2 * b]["ot"] +
                res.results[2 * b + 1]["ot"]).T + bp[None, :]
    return y
